# revision 8
# baseline (speedup 1.0000x reference)
"""Trainium2 Bass kernel for nn_CustomRNN (S=2048, B=64, I=256, H=512, E=256).

Math (per step t, carry (h_prev, m_prev), h_{-1}=h0, m_{-1}=m0):
    m_t = h_prev @ We^T + be
    h_t = tanh([x_t, m_prev] @ Wih^T + bih)
Substituting m gives a lag-2 recurrence:
    h_t = tanh(u_t + h_{t-2} @ Wc + c),   t >= 1
    h_0 = tanh(u_0 + (m0 - be) @ Wm^T + c')   [folded so U carries bias b'' = bih + be@Wm^T]
with u_t = x_t @ Wx^T (+ b''), Wc = We^T @ Wm^T (H x H), Wx = Wih[:, :I], Wm = Wih[:, I:].
Even/odd chains are independent -> process 2 steps per weight pass (N=16 moving cols).

Sharding: data-parallel over batch, 8 cores x 8 rows. All sequential work stays
on-core; no collectives. Recurrence runs with bf16 weights/state (contractive
recurrence, measured drift ~6e-3 absmax), U-GEMM in fp32r, outputs in fp32
(tanh applied straight from the fp32 PSUM pre-activation).
"""

import os
import sys

for _p in ("/opt/trn_rl_repo",):
    if _p not in sys.path and os.path.isdir(_p):
        sys.path.insert(0, _p)

from contextlib import ExitStack

import numpy as np

import concourse.bass as bass
import concourse.tile as tile
from concourse import bacc, mybir
from concourse import bass_utils
from concourse.masks import make_identity

F32 = mybir.dt.float32
F32R = mybir.dt.float32r
BF16 = mybir.dt.bfloat16
TANH = mybir.ActivationFunctionType.Tanh

N_CORES = 8
S_FULL, B_FULL, I_DIM, H_DIM, E_DIM = 2048, 64, 256, 512, 256
BC = B_FULL // N_CORES          # batch rows per core = 8
HC = H_DIM // 128               # 4 H-chunks
IC = I_DIM // 128               # 2 I-chunks
EC = E_DIM // 128               # 2 E-chunks
BLK_STEPS = 64                  # steps per U-GEMM block
BLK_PAIRS = BLK_STEPS // 2      # 32 pairs per block
OUT_GRP = 8                     # pairs per output-transpose group (16 steps)


def build_program(S=S_FULL, recur_bf16=True):
    """Build the SPMD Bass program for one core (all cores run it on their
    batch slice). Returns compiled Bacc module."""
    assert S % BLK_STEPS == 0
    n_pairs = S // 2
    n_blocks = S // BLK_STEPS

    nc = bacc.Bacc(
        "TRN2",
        target_bir_lowering=False,
        debug=False,
        enable_asserts=False,
        num_devices=N_CORES,
    )
    x_d = nc.dram_tensor("x", [S, BC, I_DIM], F32, kind="ExternalInput").ap()
    h0_d = nc.dram_tensor("h0", [BC, H_DIM], F32, kind="ExternalInput").ap()
    m0_d = nc.dram_tensor("m0", [BC, E_DIM], F32, kind="ExternalInput").ap()
    we_d = nc.dram_tensor("We", [E_DIM, H_DIM], F32, kind="ExternalInput").ap()
    wih_d = nc.dram_tensor("Wih", [H_DIM, I_DIM + E_DIM], F32, kind="ExternalInput").ap()
    be_d = nc.dram_tensor("be", [E_DIM], F32, kind="ExternalInput").ap()
    bih_d = nc.dram_tensor("bih", [H_DIM], F32, kind="ExternalInput").ap()
    out_d = nc.dram_tensor("out", [S, BC, H_DIM], F32, kind="ExternalOutput").ap()
    mt_d = nc.dram_tensor("mT", [BC, E_DIM], F32, kind="ExternalOutput").ap()

    rdt = BF16 if recur_bf16 else F32

    with tile.TileContext(nc) as tc:
        with ExitStack() as ctx:
            sing = ctx.enter_context(tc.tile_pool(name="sing", bufs=1))
            xpool = ctx.enter_context(tc.tile_pool(name="xp", bufs=2))
            xtpool = ctx.enter_context(tc.tile_pool(name="xtp", bufs=2))
            utpool = ctx.enter_context(tc.tile_pool(name="utp", bufs=2))
            stpool = ctx.enter_context(tc.tile_pool(name="stp", bufs=3))
            ringpool = ctx.enter_context(tc.tile_pool(name="ringp", bufs=2))
            hnatpool = ctx.enter_context(tc.tile_pool(name="hnatp", bufs=2))
            recps = ctx.enter_context(tc.tile_pool(name="recps", bufs=2, space="PSUM"))
            ups = ctx.enter_context(tc.tile_pool(name="ups", bufs=2, space="PSUM"))
            tpps = ctx.enter_context(tc.tile_pool(name="tpps", bufs=2, space="PSUM"))

            # ---------------- persistent tiles ----------------
            ident = sing.tile([128, 128], F32, tag="ident")
            make_identity(nc, ident[:])
            ones_row = sing.tile([1, BC], F32, tag="ones")
            nc.vector.memset(ones_row[:], 1.0)

            we_sb = sing.tile([128, EC * H_DIM], F32, tag="we")       # We rows-chunks
            wm_sb = sing.tile([128, HC * E_DIM], F32, tag="wm")       # Wm rows-chunks
            wx_sb = sing.tile([128, HC * I_DIM], F32, tag="wx")       # Wx rows-chunks
            wmT_sb = sing.tile([128, EC * H_DIM], F32, tag="wmT")     # Wm^T e-chunks
            wxT_sb = sing.tile([128, IC * H_DIM], F32R, tag="wxT")    # Wx^T i-chunks (fp32r)
            wmT_r = sing.tile([128, EC * H_DIM], F32R, tag="wmTr")    # fp32r copy for Wc gemm
            we_r = sing.tile([128, EC * H_DIM], F32R, tag="wer")      # fp32r copy of We
            weT_sb = sing.tile([128, HC * E_DIM], F32, tag="weT")     # We^T k-chunks
            wc_sb = sing.tile([128, HC * H_DIM], F32, tag="wc")       # Wc row-chunks fp32
            wcb_sb = sing.tile([128, HC * H_DIM], rdt, tag="wcb")     # Wc row-chunks bf16
            beT_sb = sing.tile([128, EC], F32, tag="beT")
            bihT_sb = sing.tile([128, HC], F32, tag="bihT")
            b2T_sb = sing.tile([128, HC], F32, tag="b2T")             # b'' = bih + be@Wm^T
            be_row = sing.tile([1, E_DIM], F32, tag="berow")
            h0_sb = sing.tile([BC, H_DIM], F32, tag="h0")
            m0_sb = sing.tile([BC, E_DIM], F32, tag="m0")
            h0T_sb = sing.tile([128, HC * BC], F32, tag="h0T")
            m0mbT_sb = sing.tile([128, EC * BC], F32, tag="m0mbT")    # (m0 - be)^T

            # ---------------- load weights/carries ----------------
            for e in range(EC):
                nc.sync.dma_start(we_sb[:, 512 * e:512 * e + 512],
                                  we_d[128 * e:128 * e + 128, :])
            for r in range(HC):
                nc.sync.dma_start(wm_sb[:, 256 * r:256 * r + 256],
                                  wih_d[128 * r:128 * r + 128, I_DIM:])
                nc.sync.dma_start(wx_sb[:, 256 * r:256 * r + 256],
                                  wih_d[128 * r:128 * r + 128, :I_DIM])
            nc.sync.dma_start(be_row[:], be_d.rearrange("(o e) -> o e", o=1))
            nc.sync.dma_start(beT_sb[:], be_d.rearrange("(c p) -> p c", p=128))
            nc.sync.dma_start(bihT_sb[:], bih_d.rearrange("(c p) -> p c", p=128))
            nc.sync.dma_start(h0_sb[:], h0_d[:, :])
            nc.sync.dma_start(m0_sb[:], m0_d[:, :])

            def pe_t(dst_ap, src_ap):
                """PE transpose src [p, n<=128] -> dst sbuf [n, p] via PSUM."""
                ps = tpps.tile([128, 128], F32, tag="tp")
                n = src_ap.shape[-1]
                p = src_ap.shape[0]
                nc.tensor.transpose(ps[:n, :p], src_ap, ident[:p, :p])
                nc.vector.tensor_copy(dst_ap, ps[:n, :p])

            # transposes of weight slices
            for r in range(HC):
                for e in range(EC):
                    pe_t(wmT_sb[:, 512 * e + 128 * r:512 * e + 128 * r + 128],
                         wm_sb[:, 256 * r + 128 * e:256 * r + 128 * e + 128])
                for i in range(IC):
                    pe_t(wxT_sb[:, 512 * i + 128 * r:512 * i + 128 * r + 128],
                         wx_sb[:, 256 * r + 128 * i:256 * r + 128 * i + 128])
            for e in range(EC):
                for k in range(HC):
                    pe_t(weT_sb[:, 256 * k + 128 * e:256 * k + 128 * e + 128],
                         we_sb[:, 512 * e + 128 * k:512 * e + 128 * k + 128])
            for k in range(HC):
                pe_t(h0T_sb[:, BC * k:BC * k + BC], h0_sb[:, 128 * k:128 * k + 128])
            for e in range(EC):
                ps = tpps.tile([128, 128], F32, tag="tp")
                nc.tensor.transpose(ps[:, :BC], m0_sb[:, 128 * e:128 * e + 128],
                                    ident[:BC, :BC])
                nc.vector.tensor_scalar(
                    m0mbT_sb[:, BC * e:BC * e + BC], ps[:, :BC],
                    beT_sb[:, e:e + 1], None, op0=mybir.AluOpType.subtract)

            # Wc = We^T @ Wm^T  (fp32r matmuls, N=512)
            nc.vector.tensor_copy(wmT_r[:], wmT_sb[:])
            nc.vector.tensor_copy(we_r[:], we_sb[:])
            for m in range(HC):
                ps = ups.tile([128, H_DIM], F32, tag="u")
                for e in range(EC):
                    nc.tensor.matmul(
                        ps[:],
                        we_r[:, 512 * e + 128 * m:512 * e + 128 * m + 128],
                        wmT_r[:, 512 * e:512 * e + 512],
                        start=(e == 0), stop=(e == EC - 1))
                nc.vector.tensor_copy(wc_sb[:, 512 * m:512 * m + 512], ps[:])
                nc.scalar.copy(wcb_sb[:, 512 * m:512 * m + 512], ps[:])

            # b''^T = bih^T + (be @ Wm^T)^T
            psc = tpps.tile([128, 128], F32, tag="tp")
            for m in range(HC):
                for e in range(EC):
                    nc.tensor.matmul(
                        psc[:, m:m + 1],
                        wmT_sb[:, 512 * e + 128 * m:512 * e + 128 * m + 128],
                        beT_sb[:, e:e + 1],
                        start=(e == 0), stop=(e == EC - 1))
            nc.vector.tensor_add(b2T_sb[:], psc[:, :HC], bihT_sb[:])

            # ---------------- U-GEMM block ----------------
            def emit_ublock(b):
                """u^T for steps [64b, 64b+64), laid out [128, HC*512] with
                col = 512*c + 16*pair_rel + 8*s + j, bias b'' added."""
                xt = xpool.tile([128, 4 * I_DIM], F32, tag="x")
                for r in range(4):
                    t0 = BLK_STEPS * b + 16 * r
                    nc.sync.dma_start(xt[:, 256 * r:256 * r + 256],
                                      x_d[t0:t0 + 16].flatten_outer_dims())
                xT = xtpool.tile([128, IC * 512], F32R, tag="xT")
                for r in range(4):
                    for i in range(IC):
                        pe_t(xT[:, 512 * i + 128 * r:512 * i + 128 * r + 128],
                             xt[:, 256 * r + 128 * i:256 * r + 128 * i + 128])
                # ut col layout: 64*pair_rel + 16*c + 8*s + j  (pair-major so the
                # inject matmul rhs is a contiguous [128, 64] slice)
                ut = utpool.tile([128, HC * 512], F32, tag="ut")
                utw = ut[:].rearrange("q (pr c u) -> q c pr u",
                                      pr=BLK_PAIRS, c=HC, u=16)
                for m in range(HC):
                    ps = ups.tile([128, 512], F32, tag="u")
                    for i in range(IC):
                        nc.tensor.matmul(
                            ps[:],
                            wxT_sb[:, 512 * i + 128 * m:512 * i + 128 * m + 128],
                            xT[:, 512 * i:512 * i + 512],
                            start=(i == 0), stop=(i == IC - 1))
                    nc.vector.tensor_scalar(
                        utw[:, m],
                        ps[:].rearrange("q (pr u) -> q pr u", pr=BLK_PAIRS, u=16),
                        b2T_sb[:, m:m + 1], None, op0=mybir.AluOpType.add)
                return ut

            # W-matmul emission order: low-k first so next pair's early
            # matmuls only need the first state ACT.
            MK_ORDER = [(m, k) for k in (0, 1) for m in range(HC)] + \
                       [(m, k) for k in (2, 3) for m in range(HC)]

            state = None      # [128, 64] rdt: col = 16c + 8s + j, holds (h_{2p-2}^T, h_{2p-1}^T)
            ring = None       # [128, OUT_GRP*64] f32, same layout per pair slot
            ut = None

            def finish_pair(p, psum):
                """tanh activations + ring writes + periodic output flush.
                ring col layout: 128*c + 16*g + 8*s + j (chunk-major so the
                output transpose input is a contiguous [128, 128] slice)."""
                nonlocal state, ring
                g = p % OUT_GRP
                if g == 0:
                    ring = ringpool.tile([128, OUT_GRP * 64], F32, tag="ring")
                new_state = stpool.tile([128, 64], rdt, tag="st")
                nc.scalar.activation(new_state[:, 0:32], psum[:, 0:32], TANH)
                nc.scalar.activation(new_state[:, 32:64], psum[:, 32:64], TANH)
                rw = ring[:].rearrange("q (c g u) -> q g c u", c=HC, g=OUT_GRP, u=16)
                nc.scalar.activation(
                    rw[:, g],
                    psum[:].rearrange("q (c u) -> q c u", c=HC, u=16), TANH)
                state = new_state
                if g == OUT_GRP - 1:
                    hnat = hnatpool.tile([128, H_DIM], F32, tag="hnat")
                    for c in range(HC):
                        ps = tpps.tile([128, 128], F32, tag="tp")
                        nc.tensor.transpose(ps[:], ring[:, 128 * c:128 * c + 128],
                                            ident[:])
                        nc.vector.tensor_copy(hnat[:, 128 * c:128 * c + 128], ps[:])
                    t0 = (p - (OUT_GRP - 1)) * 2
                    nc.sync.dma_start(out_d[t0:t0 + 16].flatten_outer_dims(), hnat[:])

            ut = emit_ublock(0)

            # ---------------- pair 0 (uses m0/h0 carries) ----------------
            psum = recps.tile([128, 64], F32, tag="rec")
            nc.tensor.matmul(psum[:], ident[:], ut[:, 0:64], start=True, stop=False)
            n_mm = EC * HC + HC * HC
            mm = 0
            for m in range(HC):
                for e in range(EC):
                    mm += 1
                    nc.tensor.matmul(
                        psum[:, 16 * m:16 * m + 8],
                        wmT_sb[:, 512 * e + 128 * m:512 * e + 128 * m + 128],
                        m0mbT_sb[:, BC * e:BC * e + BC],
                        start=False, stop=False)
            for m in range(HC):
                for k in range(HC):
                    mm += 1
                    nc.tensor.matmul(
                        psum[:, 16 * m + 8:16 * m + 16],
                        wc_sb[:, 512 * k + 128 * m:512 * k + 128 * m + 128],
                        h0T_sb[:, BC * k:BC * k + BC],
                        start=False, stop=(mm == n_mm))
            finish_pair(0, psum)

            # ---------------- steady pairs ----------------
            for p in range(1, n_pairs):
                pr = p % BLK_PAIRS
                psum = recps.tile([128, 64], F32, tag="rec")
                nc.tensor.matmul(psum[:], ident[:], ut[:, 64 * pr:64 * pr + 64],
                                 start=True, stop=False)
                for n, (m, k) in enumerate(MK_ORDER):
                    nc.tensor.matmul(
                        psum[:, 16 * m:16 * m + 16],
                        wcb_sb[:, 512 * k + 128 * m:512 * k + 128 * m + 128],
                        state[:, 16 * k:16 * k + 16],
                        start=False, stop=(n == len(MK_ORDER) - 1))
                finish_pair(p, psum)
                if pr == BLK_PAIRS - 1 and p != n_pairs - 1:
                    ut = emit_ublock(p // BLK_PAIRS + 1)

            # ---------------- epilogue: m_T = h_{S-2} @ We^T + be ----------
            ps_mt = tpps.tile([BC, E_DIM], F32, tag="mt")
            g_last = (n_pairs - 1) % OUT_GRP
            for k in range(HC):
                nc.tensor.matmul(
                    ps_mt[:],
                    ring[:, 128 * k + 16 * g_last:128 * k + 16 * g_last + BC],
                    weT_sb[:, 256 * k:256 * k + 256],
                    start=(k == 0), stop=False)
            nc.tensor.matmul(ps_mt[:], ones_row[:], be_row[:], start=False, stop=True)
            mt_sb = sing.tile([BC, E_DIM], F32, tag="mtsb")
            nc.vector.tensor_copy(mt_sb[:], ps_mt[:])
            nc.sync.dma_start(mt_d[:, :], mt_sb[:])

    nc.compile()
    return nc


_CACHE = {}


def get_program(S=S_FULL, recur_bf16=True):
    key = (S, recur_bf16)
    if key not in _CACHE:
        _CACHE[key] = build_program(S, recur_bf16)
    return _CACHE[key]


def run(inputs, S=S_FULL, recur_bf16=True, trace=False, trace_kwargs=None):
    """Shard full inputs over 8 cores, run, gather. Returns
    ((outputs, h_T, m_T), BassKernelResults)."""
    nc = get_program(S, recur_bf16)
    x = np.ascontiguousarray(np.asarray(inputs["input_seq"], dtype=np.float32))
    h0 = np.asarray(inputs["h_0"], dtype=np.float32)
    m0 = np.asarray(inputs["m_0"], dtype=np.float32)
    we = np.ascontiguousarray(np.asarray(inputs["W_embed"], dtype=np.float32))
    bih = np.asarray(inputs["b_ih"], dtype=np.float32)
    be = np.asarray(inputs["b_embed"], dtype=np.float32)
    wih = np.ascontiguousarray(np.asarray(inputs["W_ih"], dtype=np.float32))
    in_maps = []
    for cid in range(N_CORES):
        sl = slice(BC * cid, BC * cid + BC)
        in_maps.append({
            "x": np.ascontiguousarray(x[:, sl, :]),
            "h0": np.ascontiguousarray(h0[sl]),
            "m0": np.ascontiguousarray(m0[sl]),
            "We": we, "Wih": wih, "be": be, "bih": bih,
        })
    res = bass_utils.run_bass_kernel_spmd(
        nc, in_maps, core_ids=list(range(N_CORES)), trace=trace,
        **(trace_kwargs or {}))
    outs = np.concatenate([res.results[c]["out"] for c in range(N_CORES)], axis=1)
    m_t = np.concatenate([res.results[c]["mT"] for c in range(N_CORES)], axis=0)
    h_t = np.ascontiguousarray(outs[-1])
    return (outs, h_t, m_t), res


def kernel(**inputs):
    (outs, h_t, m_t), _ = run(inputs)
    return outs, h_t, m_t


# revision 10
# speedup vs baseline: 1.0327x; 1.0327x over previous
"""Trainium2 Bass kernel for nn_CustomRNN (S=2048, B=64, I=256, H=512, E=256).

Math (per step t, carry (h_prev, m_prev), h_{-1}=h0, m_{-1}=m0):
    m_t = h_prev @ We^T + be
    h_t = tanh([x_t, m_prev] @ Wih^T + bih)
Substituting m gives a lag-2 recurrence:
    h_t = tanh(u_t + h_{t-2} @ Wc + c),   t >= 1
    h_0 = tanh(u_0 + (m0 - be) @ Wm^T + c')   [folded so U carries bias b'' = bih + be@Wm^T]
with u_t = x_t @ Wx^T (+ b''), Wc = We^T @ Wm^T (H x H), Wx = Wih[:, :I], Wm = Wih[:, I:].
Even/odd chains are independent -> process 2 steps per weight pass (N=16 moving cols).

Sharding: data-parallel over batch, 8 cores x 8 rows. All sequential work stays
on-core; no collectives. Recurrence runs with bf16 weights/state (contractive
recurrence, measured drift ~6e-3 absmax), U-GEMM in fp32r, outputs in fp32
(tanh applied straight from the fp32 PSUM pre-activation).
"""

import os
import sys

for _p in ("/opt/trn_rl_repo",):
    if _p not in sys.path and os.path.isdir(_p):
        sys.path.insert(0, _p)

from contextlib import ExitStack

import numpy as np

import concourse.bass as bass
import concourse.tile as tile
from concourse import bacc, mybir
from concourse import bass_utils
from concourse.masks import make_identity

F32 = mybir.dt.float32
F32R = mybir.dt.float32r
BF16 = mybir.dt.bfloat16
TANH = mybir.ActivationFunctionType.Tanh

N_CORES = 8
S_FULL, B_FULL, I_DIM, H_DIM, E_DIM = 2048, 64, 256, 512, 256
BC = B_FULL // N_CORES          # batch rows per core = 8
HC = H_DIM // 128               # 4 H-chunks
IC = I_DIM // 128               # 2 I-chunks
EC = E_DIM // 128               # 2 E-chunks
BLK_STEPS = 64                  # steps per U-GEMM block
BLK_PAIRS = BLK_STEPS // 2      # 32 pairs per block
OUT_GRP = 8                     # pairs per output-transpose group (16 steps)


def build_program(S=S_FULL, recur_bf16=True):
    """Build the SPMD Bass program for one core (all cores run it on their
    batch slice). Returns compiled Bacc module."""
    assert S % BLK_STEPS == 0
    n_pairs = S // 2
    n_blocks = S // BLK_STEPS

    nc = bacc.Bacc(
        "TRN2",
        target_bir_lowering=False,
        debug=False,
        enable_asserts=False,
        num_devices=N_CORES,
    )
    x_d = nc.dram_tensor("x", [S, BC, I_DIM], F32, kind="ExternalInput").ap()
    h0_d = nc.dram_tensor("h0", [BC, H_DIM], F32, kind="ExternalInput").ap()
    m0_d = nc.dram_tensor("m0", [BC, E_DIM], F32, kind="ExternalInput").ap()
    we_d = nc.dram_tensor("We", [E_DIM, H_DIM], F32, kind="ExternalInput").ap()
    wih_d = nc.dram_tensor("Wih", [H_DIM, I_DIM + E_DIM], F32, kind="ExternalInput").ap()
    be_d = nc.dram_tensor("be", [E_DIM], F32, kind="ExternalInput").ap()
    bih_d = nc.dram_tensor("bih", [H_DIM], F32, kind="ExternalInput").ap()
    out_d = nc.dram_tensor("out", [S, BC, H_DIM], F32, kind="ExternalOutput").ap()
    mt_d = nc.dram_tensor("mT", [BC, E_DIM], F32, kind="ExternalOutput").ap()

    rdt = BF16 if recur_bf16 else F32

    with tile.TileContext(nc) as tc:
        with ExitStack() as ctx:
            sing = ctx.enter_context(tc.tile_pool(name="sing", bufs=1))
            xpool = ctx.enter_context(tc.tile_pool(name="xp", bufs=2))
            xtpool = ctx.enter_context(tc.tile_pool(name="xtp", bufs=2))
            utpool = ctx.enter_context(tc.tile_pool(name="utp", bufs=2))
            stpool = ctx.enter_context(tc.tile_pool(name="stp", bufs=3))
            ringpool = ctx.enter_context(tc.tile_pool(name="ringp", bufs=2))
            hnatpool = ctx.enter_context(tc.tile_pool(name="hnatp", bufs=2))
            recps = ctx.enter_context(tc.tile_pool(name="recps", bufs=2, space="PSUM"))
            ups = ctx.enter_context(tc.tile_pool(name="ups", bufs=2, space="PSUM"))
            tpps = ctx.enter_context(tc.tile_pool(name="tpps", bufs=2, space="PSUM"))

            # ---------------- persistent tiles ----------------
            ident = sing.tile([128, 128], F32, tag="ident")
            make_identity(nc, ident[:])
            ident_r = sing.tile([128, 128], F32R, tag="identr")
            nc.vector.tensor_copy(ident_r[:], ident[:])
            ones_row = sing.tile([1, BC], F32, tag="ones")
            nc.vector.memset(ones_row[:], 1.0)

            we_sb = sing.tile([128, EC * H_DIM], F32, tag="we")       # We rows-chunks
            wm_sb = sing.tile([128, HC * E_DIM], F32, tag="wm")       # Wm rows-chunks
            wx_sb = sing.tile([128, HC * I_DIM], F32, tag="wx")       # Wx rows-chunks
            wmT_sb = sing.tile([128, EC * H_DIM], F32, tag="wmT")     # Wm^T e-chunks
            wxT_sb = sing.tile([128, IC * H_DIM], F32R, tag="wxT")    # Wx^T i-chunks (fp32r)
            wmT_r = sing.tile([128, EC * H_DIM], F32R, tag="wmTr")    # fp32r copy for Wc gemm
            we_r = sing.tile([128, EC * H_DIM], F32R, tag="wer")      # fp32r copy of We
            weT_sb = sing.tile([128, HC * E_DIM], F32, tag="weT")     # We^T k-chunks
            wc_sb = sing.tile([128, HC * H_DIM], F32, tag="wc")       # Wc row-chunks fp32
            wcb_sb = sing.tile([128, HC * H_DIM], rdt, tag="wcb")     # Wc row-chunks bf16
            beT_sb = sing.tile([128, EC], F32, tag="beT")
            bihT_sb = sing.tile([128, HC], F32, tag="bihT")
            b2T_sb = sing.tile([128, HC], F32, tag="b2T")             # b'' = bih + be@Wm^T
            be_row = sing.tile([1, E_DIM], F32, tag="berow")
            h0_sb = sing.tile([BC, H_DIM], F32, tag="h0")
            m0_sb = sing.tile([BC, E_DIM], F32, tag="m0")
            h0T_sb = sing.tile([128, HC * BC], F32, tag="h0T")
            m0mbT_sb = sing.tile([128, EC * BC], F32, tag="m0mbT")    # (m0 - be)^T

            # ---------------- load weights/carries ----------------
            for e in range(EC):
                nc.sync.dma_start(we_sb[:, 512 * e:512 * e + 512],
                                  we_d[128 * e:128 * e + 128, :])
            for r in range(HC):
                nc.sync.dma_start(wm_sb[:, 256 * r:256 * r + 256],
                                  wih_d[128 * r:128 * r + 128, I_DIM:])
                nc.sync.dma_start(wx_sb[:, 256 * r:256 * r + 256],
                                  wih_d[128 * r:128 * r + 128, :I_DIM])
            nc.sync.dma_start(be_row[:], be_d.rearrange("(o e) -> o e", o=1))
            nc.sync.dma_start(beT_sb[:], be_d.rearrange("(c p) -> p c", p=128))
            nc.sync.dma_start(bihT_sb[:], bih_d.rearrange("(c p) -> p c", p=128))
            nc.sync.dma_start(h0_sb[:], h0_d[:, :])
            nc.sync.dma_start(m0_sb[:], m0_d[:, :])

            def pe_t(dst_ap, src_ap):
                """PE transpose src [p, n<=128] -> dst sbuf [n, p] via PSUM."""
                ps = tpps.tile([128, 128], F32, tag="tp")
                n = src_ap.shape[-1]
                p = src_ap.shape[0]
                nc.tensor.transpose(ps[:n, :p], src_ap, ident[:p, :p])
                nc.vector.tensor_copy(dst_ap, ps[:n, :p])

            # transposes of weight slices
            for r in range(HC):
                for e in range(EC):
                    pe_t(wmT_sb[:, 512 * e + 128 * r:512 * e + 128 * r + 128],
                         wm_sb[:, 256 * r + 128 * e:256 * r + 128 * e + 128])
                for i in range(IC):
                    pe_t(wxT_sb[:, 512 * i + 128 * r:512 * i + 128 * r + 128],
                         wx_sb[:, 256 * r + 128 * i:256 * r + 128 * i + 128])
            for e in range(EC):
                for k in range(HC):
                    pe_t(weT_sb[:, 256 * k + 128 * e:256 * k + 128 * e + 128],
                         we_sb[:, 512 * e + 128 * k:512 * e + 128 * k + 128])
            for k in range(HC):
                pe_t(h0T_sb[:, BC * k:BC * k + BC], h0_sb[:, 128 * k:128 * k + 128])
            for e in range(EC):
                ps = tpps.tile([128, 128], F32, tag="tp")
                nc.tensor.transpose(ps[:, :BC], m0_sb[:, 128 * e:128 * e + 128],
                                    ident[:BC, :BC])
                nc.vector.tensor_scalar(
                    m0mbT_sb[:, BC * e:BC * e + BC], ps[:, :BC],
                    beT_sb[:, e:e + 1], None, op0=mybir.AluOpType.subtract)

            # Wc = We^T @ Wm^T  (fp32r matmuls, N=512)
            nc.vector.tensor_copy(wmT_r[:], wmT_sb[:])
            nc.vector.tensor_copy(we_r[:], we_sb[:])
            for m in range(HC):
                ps = ups.tile([128, H_DIM], F32, tag="u")
                for e in range(EC):
                    nc.tensor.matmul(
                        ps[:],
                        we_r[:, 512 * e + 128 * m:512 * e + 128 * m + 128],
                        wmT_r[:, 512 * e:512 * e + 512],
                        start=(e == 0), stop=(e == EC - 1))
                nc.vector.tensor_copy(wc_sb[:, 512 * m:512 * m + 512], ps[:])
                nc.scalar.copy(wcb_sb[:, 512 * m:512 * m + 512], ps[:])

            # b''^T = bih^T + (be @ Wm^T)^T
            psc = tpps.tile([128, 128], F32, tag="tp")
            for m in range(HC):
                for e in range(EC):
                    nc.tensor.matmul(
                        psc[:, m:m + 1],
                        wmT_sb[:, 512 * e + 128 * m:512 * e + 128 * m + 128],
                        beT_sb[:, e:e + 1],
                        start=(e == 0), stop=(e == EC - 1))
            nc.vector.tensor_add(b2T_sb[:], psc[:, :HC], bihT_sb[:])

            # ---------------- U-GEMM block ----------------
            def emit_ublock(b):
                """u^T for steps [64b, 64b+64), laid out [128, HC*512] with
                col = 512*c + 16*pair_rel + 8*s + j, bias b'' added."""
                xt = xpool.tile([128, 4 * I_DIM], F32, tag="x")
                for r in range(4):
                    t0 = BLK_STEPS * b + 16 * r
                    nc.sync.dma_start(xt[:, 256 * r:256 * r + 256],
                                      x_d[t0:t0 + 16].flatten_outer_dims())
                xT = xtpool.tile([128, IC * 512], F32R, tag="xT")
                for r in range(4):
                    for i in range(IC):
                        pe_t(xT[:, 512 * i + 128 * r:512 * i + 128 * r + 128],
                             xt[:, 256 * r + 128 * i:256 * r + 128 * i + 128])
                # ut col layout: 64*pair_rel + 16*c + 8*s + j  (pair-major so the
                # inject matmul rhs is a contiguous [128, 64] slice)
                ut = utpool.tile([128, HC * 512], F32R, tag="ut")
                utw = ut[:].rearrange("q (pr c u) -> q c pr u",
                                      pr=BLK_PAIRS, c=HC, u=16)
                for m in range(HC):
                    ps = ups.tile([128, 512], F32, tag="u")
                    for i in range(IC):
                        nc.tensor.matmul(
                            ps[:],
                            wxT_sb[:, 512 * i + 128 * m:512 * i + 128 * m + 128],
                            xT[:, 512 * i:512 * i + 512],
                            start=(i == 0), stop=(i == IC - 1))
                    nc.vector.tensor_scalar(
                        utw[:, m],
                        ps[:].rearrange("q (pr u) -> q pr u", pr=BLK_PAIRS, u=16),
                        b2T_sb[:, m:m + 1], None, op0=mybir.AluOpType.add)
                return ut

            # W-matmul emission order: low-k first so next pair's early
            # matmuls only need the first state ACT.
            MK_ORDER = [(m, k) for k in (0, 1) for m in range(HC)] + \
                       [(0, 2), (1, 2), (0, 3), (1, 3),
                        (2, 2), (3, 2), (2, 3), (3, 3)]

            state = None      # [128, 64] rdt: col = 16c + 8s + j, holds (h_{2p-2}^T, h_{2p-1}^T)
            ring = None       # [128, OUT_GRP*64] f32, same layout per pair slot
            ut = None

            def finish_pair(p, psum):
                """tanh activations + ring writes + periodic output flush.
                ring col layout: 128*c + 16*g + 8*s + j (chunk-major so the
                output transpose input is a contiguous [128, 128] slice)."""
                nonlocal state, ring
                g = p % OUT_GRP
                if g == 0:
                    ring = ringpool.tile([128, OUT_GRP * 64], F32, tag="ring")
                new_state = stpool.tile([128, 64], rdt, tag="st")
                nc.scalar.activation(new_state[:, 0:32], psum[:, 0:32], TANH)
                nc.scalar.activation(new_state[:, 32:64], psum[:, 32:64], TANH)
                rw = ring[:].rearrange("q (c g u) -> q g c u", c=HC, g=OUT_GRP, u=16)
                nc.scalar.activation(
                    rw[:, g],
                    psum[:].rearrange("q (c u) -> q c u", c=HC, u=16), TANH)
                state = new_state
                if g == OUT_GRP - 1:
                    hnat = hnatpool.tile([128, H_DIM], F32, tag="hnat")
                    for c in range(HC):
                        ps = tpps.tile([128, 128], F32, tag="tp")
                        nc.tensor.transpose(ps[:], ring[:, 128 * c:128 * c + 128],
                                            ident[:])
                        nc.vector.tensor_copy(hnat[:, 128 * c:128 * c + 128], ps[:])
                    t0 = (p - (OUT_GRP - 1)) * 2
                    nc.sync.dma_start(out_d[t0:t0 + 16].flatten_outer_dims(), hnat[:])

            ut = emit_ublock(0)

            # ---------------- pair 0 (uses m0/h0 carries) ----------------
            psum = recps.tile([128, 64], F32, tag="rec")
            nc.tensor.matmul(psum[:], ident_r[:], ut[:, 0:64], start=True, stop=False)
            n_mm = EC * HC + HC * HC
            mm = 0
            for m in range(HC):
                for e in range(EC):
                    mm += 1
                    nc.tensor.matmul(
                        psum[:, 16 * m:16 * m + 8],
                        wmT_sb[:, 512 * e + 128 * m:512 * e + 128 * m + 128],
                        m0mbT_sb[:, BC * e:BC * e + BC],
                        start=False, stop=False)
            for m in range(HC):
                for k in range(HC):
                    mm += 1
                    nc.tensor.matmul(
                        psum[:, 16 * m + 8:16 * m + 16],
                        wc_sb[:, 512 * k + 128 * m:512 * k + 128 * m + 128],
                        h0T_sb[:, BC * k:BC * k + BC],
                        start=False, stop=(mm == n_mm))
            finish_pair(0, psum)

            # ---------------- steady pairs ----------------
            for p in range(1, n_pairs):
                pr = p % BLK_PAIRS
                psum = recps.tile([128, 64], F32, tag="rec")
                nc.tensor.matmul(psum[:], ident_r[:], ut[:, 64 * pr:64 * pr + 64],
                                 start=True, stop=False)
                for n, (m, k) in enumerate(MK_ORDER):
                    nc.tensor.matmul(
                        psum[:, 16 * m:16 * m + 16],
                        wcb_sb[:, 512 * k + 128 * m:512 * k + 128 * m + 128],
                        state[:, 16 * k:16 * k + 16],
                        start=False, stop=(n == len(MK_ORDER) - 1))
                finish_pair(p, psum)
                if pr == BLK_PAIRS - 1 and p != n_pairs - 1:
                    ut = emit_ublock(p // BLK_PAIRS + 1)

            # ---------------- epilogue: m_T = h_{S-2} @ We^T + be ----------
            ps_mt = tpps.tile([BC, E_DIM], F32, tag="mt")
            g_last = (n_pairs - 1) % OUT_GRP
            for k in range(HC):
                nc.tensor.matmul(
                    ps_mt[:],
                    ring[:, 128 * k + 16 * g_last:128 * k + 16 * g_last + BC],
                    weT_sb[:, 256 * k:256 * k + 256],
                    start=(k == 0), stop=False)
            nc.tensor.matmul(ps_mt[:], ones_row[:], be_row[:], start=False, stop=True)
            mt_sb = sing.tile([BC, E_DIM], F32, tag="mtsb")
            nc.vector.tensor_copy(mt_sb[:], ps_mt[:])
            nc.sync.dma_start(mt_d[:, :], mt_sb[:])

    nc.compile()
    return nc


_CACHE = {}


def get_program(S=S_FULL, recur_bf16=True):
    key = (S, recur_bf16)
    if key not in _CACHE:
        _CACHE[key] = build_program(S, recur_bf16)
    return _CACHE[key]


def run(inputs, S=S_FULL, recur_bf16=True, trace=False, trace_kwargs=None):
    """Shard full inputs over 8 cores, run, gather. Returns
    ((outputs, h_T, m_T), BassKernelResults)."""
    nc = get_program(S, recur_bf16)
    x = np.ascontiguousarray(np.asarray(inputs["input_seq"], dtype=np.float32))
    h0 = np.asarray(inputs["h_0"], dtype=np.float32)
    m0 = np.asarray(inputs["m_0"], dtype=np.float32)
    we = np.ascontiguousarray(np.asarray(inputs["W_embed"], dtype=np.float32))
    bih = np.asarray(inputs["b_ih"], dtype=np.float32)
    be = np.asarray(inputs["b_embed"], dtype=np.float32)
    wih = np.ascontiguousarray(np.asarray(inputs["W_ih"], dtype=np.float32))
    in_maps = []
    for cid in range(N_CORES):
        sl = slice(BC * cid, BC * cid + BC)
        in_maps.append({
            "x": np.ascontiguousarray(x[:, sl, :]),
            "h0": np.ascontiguousarray(h0[sl]),
            "m0": np.ascontiguousarray(m0[sl]),
            "We": we, "Wih": wih, "be": be, "bih": bih,
        })
    res = bass_utils.run_bass_kernel_spmd(
        nc, in_maps, core_ids=list(range(N_CORES)), trace=trace,
        **(trace_kwargs or {}))
    outs = np.concatenate([res.results[c]["out"] for c in range(N_CORES)], axis=1)
    m_t = np.concatenate([res.results[c]["mT"] for c in range(N_CORES)], axis=0)
    h_t = np.ascontiguousarray(outs[-1])
    return (outs, h_t, m_t), res


def kernel(**inputs):
    (outs, h_t, m_t), _ = run(inputs)
    return outs, h_t, m_t
